# revision 1
# baseline (speedup 1.0000x reference)
"""3-layer GCN block (improved gcn_norm, identity activations, residuals)
on 8 Trainium2 NeuronCores.

Strategy (graph/data parallel, dst-sharded):
  - Nodes are permuted into 784 tiles of 128 (serpentine bin-packing on
    in-degree to balance per-tile edge counts); 98 tiles per core.
  - Aggregation commutes with the dense transform:  Ahat(X W) = (Ahat X) W,
    so each core aggregates raw features for its 12544-node shard and then
    applies the 128x128 weight to the shard only.
  - The gather table holds t[u] = bf16(out[u] * dinv[u]); per-edge weights
    then factor as norm_e = dinv[dst] * (t-scale), so selection matrices are
    exact 0/1 one-hots, self-loops contribute 2*t[u] (a constant 2I matmul
    fed by a descriptor-free contiguous DMA of the core's own shard), and
    dinv[dst] is folded into the epilogue.
  - Real edges are packed into 128-edge chunks per (2-tile window, source
    range) cell; 5 ranges of 20071 rows keep int16 gather indices in range.
    The per-cell chunk count is the max over the 8 cores (baked into the
    SPMD program), so padding is ~10% instead of the ~25% a per-(tile,
    range) split costs.  dma_gather calls carry <=8 chunks (1024-descriptor
    ucode limit; measured Pool cost ~216ns + ~2ns/descriptor per call, and
    each of the 4 SWDGE queues drains at ~30GB/s, so call size beyond 8
    chunks is moot) and round-robin the 4 SWDGE queues.
  - A DVE-built one-hot SEL[e, d] = (dst_window_local_e == d) (256 wide)
    turns the segment-sum into PSUM-accumulated bf16 matmuls over a 256-col
    window:  aggT[feat, dst] += msg[e, feat].T @ SEL.  Each 4-tile group
    shares one PSUM bank ([128, 512] f32); only the bank's first matmul
    uses start=True (start clears the has_written bits bank-wide), the
    bank's four self-loop matmuls then overwrite-and-mark their quarter,
    and every chunk matmul accumulates.  The PE executes in program order,
    which makes that sequencing sound.
  - aggT is already the lhsT layout for the fp32 weight matmul
    out = aggT.T @ W, then + bias + residual(s) (all fp32).
  - Between layers the 8 bf16 shards are exchanged with an AllGather into a
    Shared DRAM buffer that next layer's gathers read; the fp32 residual
    stays core-local.
"""
import math
import numpy as np

P = 128
D = 128
NCORES = 8
W2 = 2          # tiles per destination window
NR = 5          # source ranges


class _Cfg:
    def __init__(self, n_nodes, tiles_per_core=98, group_t=4):
        self.N = n_nodes
        self.TPC = tiles_per_core
        self.SHARD = tiles_per_core * P
        self.NPAD = NCORES * self.SHARD
        self.NT = NCORES * tiles_per_core          # global tiles
        self.NWC = tiles_per_core // W2            # windows per core
        self.RSZ = -(-self.NPAD // NR)
        assert self.RSZ <= 32767, "int16 gather index range exceeded"
        gs = [group_t] * (tiles_per_core // group_t)
        if tiles_per_core % group_t:
            gs.append(tiles_per_core % group_t)
        self.GS = gs


CFG = _Cfg(100000)


def _host_prep(edge_index, cfg, seed0=0):
    """Permute nodes; pack real edges into core-uniform chunk slots."""
    import ml_dtypes

    N, NPAD, NT, TPC = cfg.N, cfg.NPAD, cfg.NT, cfg.TPC
    RSZ, NWC = cfg.RSZ, cfg.NWC

    src = edge_index[0].astype(np.int64)
    dst = edge_index[1].astype(np.int64)

    indeg = np.bincount(dst, minlength=N)
    deg = (indeg + 2).astype(np.float32)
    dinv = (1.0 / np.sqrt(deg)).astype(np.float32)

    # ---- node -> padded id (tile*128 + slot) via serpentine on in-degree,
    # choosing the seed that minimises total chunk slots ----
    Lall = np.zeros(NPAD, np.float64)
    Lall[:N] = indeg + 1
    best = None
    for attempt in range(6):
        rng = np.random.default_rng(seed0 + attempt)
        order = np.argsort(-(Lall + rng.random(NPAD)), kind="stable")
        ranks = np.empty(NPAD, np.int64)
        ranks[order] = np.arange(NPAD)
        blk, j = ranks // NT, ranks % NT
        tile = np.where(blk % 2 == 0, j, NT - 1 - j)
        cand = tile * P + blk  # each block contributes one node per tile
        t_e = cand[dst] // P
        cell = ((t_e // TPC) * NWC + (t_e % TPC) // W2) * NR + cand[src] // RSZ
        counts = np.bincount(cell, minlength=NCORES * NWC * NR)
        K = -(-counts.reshape(NCORES, NWC, NR).max(axis=0) // P)
        slots = int(K.sum())
        if best is None or slots < best[0]:
            best = (slots, cand, K)
    _, pid, K = best

    # ---- pack real edges into slots ----
    e_src = pid[src]
    e_dst = pid[dst]
    t_e = e_dst // P
    core = t_e // TPC
    tl_e = t_e % TPC
    w_e = tl_e // W2
    par_e = tl_e % W2
    r_e = e_src // RSZ

    # column bases in device iteration order: group-major, then range,
    # then window within group
    GS = cfg.GS
    colbase = np.zeros((NWC, NR), np.int64)
    acc = 0
    t0 = 0
    for T in GS:
        w0 = t0 // W2
        nw = T // W2 if T % W2 == 0 else (T + W2 - 1) // W2
        for r in range(NR):
            for wi in range(nw):
                colbase[w0 + wi, r] = acc
                acc += K[w0 + wi, r]
        t0 += T
    COLS = acc
    ICOLS = 8 * COLS

    cell = (core * NWC + w_e) * NR + r_e
    ordr = np.argsort(cell, kind="stable")
    cell_s = cell[ordr]
    counts = np.bincount(cell, minlength=NCORES * NWC * NR)
    starts = np.zeros(NCORES * NWC * NR + 1, np.int64)
    np.cumsum(counts, out=starts[1:])
    i_in = np.arange(cell_s.shape[0]) - starts[cell_s]

    es, ed = e_src[ordr], e_dst[ordr]
    cr, wr, rr, pr = core[ordr], w_e[ordr], r_e[ordr], par_e[ordr]
    qk = i_in // P
    pk = i_in % P
    assert (qk < K[wr, rr]).all()
    col = colbase[wr, rr] + qk

    dstsel = np.full((NCORES, P, COLS), 999.0, np.float32)  # pads -> SEL 0
    idxs16 = np.zeros((NCORES, 16, ICOLS), np.int16)        # pads -> row 0

    flat = (cr * P + pk) * COLS + col
    dstsel.reshape(-1)[flat] = (pr * P + ed % P).astype(np.float32)
    icol = col * 8 + pk // 16
    iflat = (cr * 16 + pk % 16) * ICOLS + icol
    idxs16.reshape(-1)[iflat] = (es - rr * RSZ).astype(np.int16)

    idxs16 = np.tile(idxs16, (1, 8, 1))    # replicate to 128 partitions

    dinv_pad = np.zeros(NPAD, np.float32)
    dinv_pad[pid[:N]] = dinv
    # [core][128, TPC]: column t = dinv of tile t's 128 nodes
    dinv_tiles = np.ascontiguousarray(
        dinv_pad.reshape(NCORES, TPC, P).transpose(0, 2, 1))

    return dict(
        K=K, colbase=colbase, COLS=COLS, pid=pid, dinv_pad=dinv_pad,
        dinv_tiles=dinv_tiles, dstsel=dstsel, idxs16=idxs16,
        bf16=ml_dtypes.bfloat16,
    )


# ------------------------------------------------------------------ device --

_NC_CACHE = {}


def _build_nc(cfg, K, colbase, nlayers=3):
    key = (cfg.N, cfg.TPC, K.tobytes(), nlayers)
    if key in _NC_CACHE:
        return _NC_CACHE[key]

    import concourse.bacc as bacc
    import concourse.mybir as mybir
    import concourse.tile as tile

    NPAD, SHARD, TPC, RSZ, GS = cfg.NPAD, cfg.SHARD, cfg.TPC, cfg.RSZ, cfg.GS
    COLS = int(K.sum())
    ICOLS = 8 * COLS
    f32 = mybir.dt.float32
    bf16 = mybir.dt.bfloat16

    MAXCH = 8
    nc = bacc.Bacc("TRN2", target_bir_lowering=False, debug=False,
                   num_devices=NCORES, num_swdge_queues=4,
                   dynamic_dma_scratch_size=49152)

    # full bf16 pre-scaled table for layer-0 gathers (same array every core)
    tfull0 = nc.dram_tensor("tfull0", [NPAD, D], bf16, kind="ExternalInput")
    # this core's shard of it (for the self-loop chunk)
    tsh0 = nc.dram_tensor("tsh0", [SHARD, D], bf16, kind="ExternalInput")
    # fp32 residual shard (= x rows of this core's shard)
    xsh = nc.dram_tensor("xsh", [SHARD, D], f32, kind="ExternalInput")
    idxs = nc.dram_tensor("idxs", [P, ICOLS], mybir.dt.int16, kind="ExternalInput")
    dstsel = nc.dram_tensor("dstsel", [P, COLS], f32, kind="ExternalInput")
    dinvt = nc.dram_tensor("dinvt", [P, TPC], f32, kind="ExternalInput")
    Ws = [nc.dram_tensor(f"W{l}", [D, D], f32, kind="ExternalInput") for l in range(3)]
    brs = [nc.dram_tensor(f"br{l}", [P, D], f32, kind="ExternalInput") for l in range(3)]
    ysh = nc.dram_tensor("ysh", [SHARD, D], f32, kind="ExternalOutput")

    # bf16 scaled shards produced per layer (AllGather inputs + self rows)
    agin = [nc.dram_tensor(f"agin{l}", [SHARD, D], bf16) for l in range(2)]
    ofull = [nc.dram_tensor(f"ofull{l}", [NPAD, D], bf16, addr_space="Shared")
             for l in range(2)]
    # fp32 residual buffers for layers 1, 2
    res = [nc.dram_tensor(f"res{l}", [SHARD, D], f32) for l in range(2)]

    with tile.TileContext(nc) as tc:
        with (
            tc.tile_pool(name="const", bufs=1) as cp,
            tc.tile_pool(name="gath", bufs=10) as gp,
            tc.tile_pool(name="selp", bufs=4) as sp,
            tc.tile_pool(name="work", bufs=3) as wp,
            tc.tile_pool(name="pag", bufs=6, space="PSUM") as pag,
            tc.tile_pool(name="pout", bufs=2, space="PSUM") as pout,
        ):
            # --- constants ---
            idx_sb = cp.tile([P, ICOLS], mybir.dt.int16)
            nc.sync.dma_start(idx_sb[:], idxs.ap())
            ds_sb = cp.tile([P, COLS], f32)
            nc.sync.dma_start(ds_sb[:], dstsel.ap())
            dv_sb = cp.tile([P, TPC], f32)
            nc.sync.dma_start(dv_sb[:], dinvt.ap())
            W_sb = []
            b_sb = []
            for l in range(3):
                t = cp.tile([D, D], f32, tag=f"W{l}")
                nc.sync.dma_start(t[:], Ws[l].ap())
                W_sb.append(t)
                t = cp.tile([P, D], f32, tag=f"br{l}")
                nc.sync.dma_start(t[:], brs[l].ap())
                b_sb.append(t)
            iota_i = cp.tile([P, P], mybir.dt.int32)
            nc.gpsimd.iota(iota_i[:], pattern=[[1, P]], base=0, channel_multiplier=0)
            iota_f = cp.tile([P, P], f32)
            nc.vector.tensor_copy(iota_f[:], iota_i[:])
            iotac_i = cp.tile([P, 1], mybir.dt.int32)
            nc.gpsimd.iota(iotac_i[:], pattern=[[0, 1]], base=0, channel_multiplier=1)
            iotac_f = cp.tile([P, 1], f32)
            nc.vector.tensor_copy(iotac_f[:], iotac_i[:])
            eye2 = cp.tile([P, P], bf16)
            nc.vector.tensor_scalar(
                out=eye2[:], in0=iota_f[:],
                scalar1=iotac_f[:], scalar2=2.0,
                op0=mybir.AluOpType.is_equal, op1=mybir.AluOpType.mult)
            iota2_i = cp.tile([P, 2 * P], mybir.dt.int32)
            nc.gpsimd.iota(iota2_i[:], pattern=[[1, 2 * P]], base=0,
                           channel_multiplier=0)
            iota2_f = cp.tile([P, 2 * P], f32)
            nc.vector.tensor_copy(iota2_f[:], iota2_i[:])

            qrr = [0]
            for layer in range(nlayers):
                gsrc = [tfull0, ofull[0], ofull[1]][layer]
                selfsrc = [tsh0, agin[0], agin[1]][layer]
                resid = [xsh, res[0], res[1]][layer]
                res_next = [res[0], res[1], None][layer]
                out_t = ysh if layer == nlayers - 1 else None
                with nc.named_scope(f"layer{layer}"):
                    t0 = 0
                    for g, T in enumerate(GS):
                        w0 = t0 // W2
                        nw = (T + W2 - 1) // W2
                        # one PSUM bank holds the whole 4-tile group
                        psb = pag.tile([P, 4 * P], f32, tag="agg",
                                       name=f"ps_l{layer}_g{g}")
                        # self-loop chunks: contiguous shard rows, 2I matmul.
                        # Only the bank's FIRST matmul may use start=True
                        # (start clears has_written bits bank-wide); the
                        # other selfs overwrite their cleared quarter.
                        for tl in range(T):
                            t = t0 + tl
                            selfr = wp.tile([P, P], bf16, tag="selfr",
                                            name="selfr")
                            nc.sync.dma_start(
                                selfr[:], selfsrc.ap()[t * P:(t + 1) * P, :])
                            nc.tensor.matmul(
                                out=psb[:, tl * P:(tl + 1) * P],
                                lhsT=selfr[:], rhs=eye2[:],
                                start=(tl == 0), stop=False,
                                skip_group_check=True)
                        for r in range(NR):
                            # chunk -> window map for this (group, range)
                            wl_of = []
                            for wi in range(nw):
                                wl_of += [wi] * int(K[w0 + wi, r])
                            n_gr = len(wl_of)
                            cbase = int(colbase[w0, r])
                            k0 = 0
                            while k0 < n_gr:
                                nch = min(MAXCH, n_gr - k0)
                                gt = gp.tile([P, nch, P], bf16, tag="gath",
                                             name="gt")
                                nc.gpsimd.dma_gather(
                                    out_ap=gt[:],
                                    in_ap=gsrc.ap()[r * RSZ:
                                                    min((r + 1) * RSZ, NPAD), :],
                                    idxs_ap=idx_sb[:, (cbase + k0) * 8:
                                                   (cbase + k0 + nch) * 8],
                                    num_idxs=nch * P,
                                    num_idxs_reg=nch * P,
                                    elem_size=D,
                                    elem_step=D,
                                    queue_num=qrr[0] % 4,
                                )
                                qrr[0] += 1
                                selb = sp.tile([P, nch, 2 * P], bf16,
                                               tag="sel", name="selb")
                                nc.vector.tensor_tensor(
                                    out=selb[:],
                                    in0=iota2_f[:].rearrange(
                                        "p (c m) -> p c m", c=1
                                    ).to_broadcast([P, nch, 2 * P]),
                                    in1=ds_sb[:, cbase + k0:cbase + k0 + nch]
                                    .rearrange("p (c m) -> p c m", m=1)
                                    .to_broadcast([P, nch, 2 * P]),
                                    op=mybir.AluOpType.is_equal,
                                )
                                for kk in range(nch):
                                    wl = wl_of[k0 + kk]
                                    last = (r == NR - 1 and
                                            k0 + kk == n_gr - 1)
                                    nc.tensor.matmul(
                                        out=psb[:, wl * 2 * P:
                                                (wl + 1) * 2 * P],
                                        lhsT=gt[:, kk, :],
                                        rhs=selb[:, kk, :],
                                        start=False,
                                        stop=last,
                                        skip_group_check=True,
                                    )
                                k0 += nch
                        for tl in range(T):
                            t = t0 + tl
                            aggT = wp.tile([P, P], f32, tag="aggT", name="aggT")
                            nc.scalar.activation(
                                out=aggT[:], in_=psb[:, tl * P:(tl + 1) * P],
                                func=mybir.ActivationFunctionType.Copy)
                            old = wp.tile([P, P], f32, tag="old", name="old")
                            nc.sync.dma_start(
                                old[:], resid.ap()[t * P:(t + 1) * P, :])
                            pso = pout.tile([P, P], f32, tag="out", name="pso")
                            nc.tensor.matmul(out=pso[:], lhsT=aggT[:],
                                             rhs=W_sb[layer][:],
                                             start=True, stop=True)
                            outn = wp.tile([P, P], f32, tag="outn", name="outn")
                            nc.vector.tensor_scalar(
                                out=outn[:], in0=pso[:],
                                scalar1=dv_sb[:, t:t + 1], scalar2=None,
                                op0=mybir.AluOpType.mult)
                            nc.vector.tensor_add(outn[:], outn[:], old[:])
                            nc.vector.tensor_add(outn[:], outn[:], b_sb[layer][:])
                            if layer == 2 and nlayers == 3:
                                xres = wp.tile([P, P], f32, tag="xres",
                                               name="xres")
                                nc.sync.dma_start(
                                    xres[:], xsh.ap()[t * P:(t + 1) * P, :])
                                nc.vector.tensor_add(outn[:], outn[:], xres[:])
                            if out_t is not None:
                                nc.sync.dma_start(
                                    out_t.ap()[t * P:(t + 1) * P, :], outn[:])
                            else:
                                nc.sync.dma_start(
                                    res_next.ap()[t * P:(t + 1) * P, :], outn[:])
                                scl = wp.tile([P, P], bf16, tag="scl",
                                              name="scl")
                                nc.scalar.activation(
                                    out=scl[:], in_=outn[:],
                                    func=mybir.ActivationFunctionType.Copy,
                                    scale=dv_sb[:, t:t + 1])
                                nc.sync.dma_start(
                                    agin[layer].ap()[t * P:(t + 1) * P, :],
                                    scl[:])
                        t0 += T
                if layer < nlayers - 1:
                    with nc.named_scope(f"ag{layer}"):
                        nc.gpsimd.collective_compute(
                            "AllGather",
                            mybir.AluOpType.bypass,
                            replica_groups=[list(range(NCORES))],
                            ins=[agin[layer].ap()],
                            outs=[ofull[layer].ap()],
                        )
    nc.compile()
    _NC_CACHE[key] = nc
    return nc


def _make_in_maps(prep, x, W0, b0, W1, b1, W2, b2, cfg):
    bf16 = prep["bf16"]
    x = np.asarray(x, np.float32)
    x_pad = np.zeros((cfg.NPAD, D), np.float32)
    x_pad[prep["pid"][:cfg.N]] = x
    t0 = (x_pad * prep["dinv_pad"][:, None]).astype(bf16)

    bl = [np.broadcast_to(np.asarray(b, np.float32), (P, D)).copy()
          for b in (b0, b1, b2)]
    Wl = [np.ascontiguousarray(np.asarray(w, np.float32)) for w in (W0, W1, W2)]
    maps = []
    for k in range(NCORES):
        sl = slice(k * cfg.SHARD, (k + 1) * cfg.SHARD)
        m = {
            "tfull0": t0,
            "tsh0": np.ascontiguousarray(t0[sl]),
            "xsh": np.ascontiguousarray(x_pad[sl]),
            "idxs": np.ascontiguousarray(prep["idxs16"][k]),
            "dstsel": np.ascontiguousarray(prep["dstsel"][k]),
            "dinvt": np.ascontiguousarray(prep["dinv_tiles"][k]),
        }
        for l in range(3):
            m[f"W{l}"] = Wl[l]
            m[f"br{l}"] = bl[l]
        maps.append(m)
    return maps


_PREP_CACHE = {}


def _run(x, edge_index, W0, b0, W1, b1, W2, b2, cfg, trace=False, nlayers=3):
    from concourse.bass_utils import run_bass_kernel_spmd

    edge_index = np.asarray(edge_index)
    key = (edge_index.tobytes()[:4096], edge_index.shape,
           int(edge_index[:, ::997].sum()))
    if key in _PREP_CACHE:
        prep = _PREP_CACHE[key]
    else:
        prep = _host_prep(edge_index, cfg)
        _PREP_CACHE.clear()
        _PREP_CACHE[key] = prep

    nc = _build_nc(cfg, prep["K"], prep["colbase"], nlayers=nlayers)
    in_maps = _make_in_maps(prep, x, W0, b0, W1, b1, W2, b2, cfg)
    res = run_bass_kernel_spmd(
        nc, in_maps, core_ids=list(range(NCORES)), trace=trace)
    ysh = np.concatenate([res.results[k]["ysh"] for k in range(NCORES)], axis=0)
    y = ysh[prep["pid"][:cfg.N]]
    return y, res


def kernel(x, edge_index, W0, b0, W1, b1, W2, b2):
    y, _ = _run(x, edge_index, W0, b0, W1, b1, W2, b2, CFG, trace=False)
    return y

